# revision 1
# baseline (speedup 1.0000x reference)
"""Routed-LoRA linear layer (moe_routing) on 8 trn2 NeuronCores.

Math (per token t):
  out[t, :] = W @ x[t] + b + 2.0 * sum_n mask[n, t] * (B_n @ (A_n @ x[t]))

Strategy:
  - Data-parallel over B*T = 65536 tokens: 8192 tokens per core.
  - All operand transposes are done host-side (numpy marshaling) so the
    device only ever streams contiguous, partition-friendly layouts:
      xt  [D_IN, TOK]  = x-shard transposed      (contraction dim major)
      wt  [D_IN, D_OUT] = W.T
      at  [D_IN, NR]    = fused-A.T
      btr [NR, D_OUT]   = fused-B.T
      msk [NR, TOK]     = routing mask expanded to rank dim, pre-scaled
  - fp32r matmuls (full PE rate at N=512), LoRA delta accumulated into the
    same PSUM bank as the base matmul, bias added during PSUM->SBUF copy.
"""

import numpy as np

import concourse.bass as bass
from concourse import bacc
import concourse.mybir as mybir
import concourse.tile as tile
from concourse.bass_utils import run_bass_kernel_spmd

N_CORES = 8
B, T = 8, 8192
D_IN = 1024
D_OUT = 1024
N_ADAPT, R = 4, 16
NR = N_ADAPT * R  # 64
SCALING = 32.0 / 16.0

TOK = B * T // N_CORES  # 8192 tokens per core
SUP = 512               # tokens per supertile
N_SUP = TOK // SUP      # 16
SUB = 128               # tokens per matmul M-tile
N_SUB = SUP // SUB      # 4
P = 128
KC = D_IN // P          # 8 contraction chunks
NB = D_OUT // 512       # 2 PSUM-bank column halves

F32 = mybir.dt.float32
F32R = mybir.dt.float32r


def build_bass(nrep=1, xp_bufs=3, pso_bufs=2, n_inner=False, split_bias=False):
    nc = bacc.Bacc(
        "TRN2", target_bir_lowering=False, debug=False, num_devices=N_CORES
    )

    xt_d = nc.dram_tensor("xt", [D_IN, TOK], F32R, kind="ExternalInput")
    wt_d = nc.dram_tensor("wt", [D_IN, D_OUT], F32R, kind="ExternalInput")
    at_d = nc.dram_tensor("at", [D_IN, NR], F32R, kind="ExternalInput")
    bt_d = nc.dram_tensor("btr", [NR, D_OUT], F32R, kind="ExternalInput")
    bias_d = nc.dram_tensor("bias", [D_OUT], F32, kind="ExternalInput")
    msk_d = nc.dram_tensor("msk", [NR, TOK], F32, kind="ExternalInput")
    out_d = nc.dram_tensor("out", [TOK, D_OUT], F32, kind="ExternalOutput")

    xt_r = xt_d.ap().rearrange("(kc p) t -> p kc t", p=P)
    wt_r = wt_d.ap().rearrange("(kc p) n -> p kc n", p=P)
    at_r = at_d.ap().rearrange("(kc p) j -> p kc j", p=P)
    out_r = out_d.ap().rearrange("(s q p) n -> s p q n", q=N_SUB, p=P)
    bias_bcast = bass.AP(
        tensor=bias_d, offset=0, ap=[[0, P], [1, D_OUT]]
    )

    with tile.TileContext(nc) as tc:
        with (
            tc.tile_pool(name="const", bufs=1) as const,
            tc.tile_pool(name="xp", bufs=xp_bufs) as xp,
            tc.tile_pool(name="sp", bufs=2) as sp,
            tc.tile_pool(name="op", bufs=2) as op,
            tc.tile_pool(name="pss", bufs=2, space="PSUM") as pss,
            tc.tile_pool(name="pso", bufs=pso_bufs, space="PSUM") as pso,
        ):
            w_sb = const.tile([P, KC, D_OUT], F32R)
            a_sb = const.tile([P, KC, NR], F32R)
            bt_sb = const.tile([NR, D_OUT], F32R)
            b_sb = const.tile([P, D_OUT], F32)
            m_sb = const.tile([NR, TOK], F32)
            # Preload order matters for startup latency: the first s-pass
            # matmuls need a_sb + x0 (sync queue), the first main matmuls
            # need w chunk 0 (scalar queue, split per-k so MMs start after
            # ~1.4us instead of waiting for the whole 4MB W load).
            nc.sync.dma_start(out=a_sb[:], in_=at_r)
            nc.scalar.dma_start(out=bt_sb[:], in_=bt_d.ap())
            for k in range(KC):
                nc.scalar.dma_start(out=w_sb[:, k, :], in_=wt_r[:, k, :])
            nc.gpsimd.dma_start(out=b_sb[:], in_=bias_bcast)

            for _rep in range(nrep):
                for s in range(N_SUP):
                    t0 = s * SUP
                    x_sb = xp.tile([P, KC, SUP], F32R, tag="x")
                    nc.sync.dma_start(
                        out=x_sb[:], in_=xt_r[:, :, t0 : t0 + SUP]
                    )
                    if _rep == 0:
                        # stream the mask in per-supertile so the first
                        # mask-multiply isn't gated on a monolithic 2MB load
                        nc.sync.dma_start(
                            out=m_sb[:, t0 : t0 + SUP],
                            in_=msk_d.ap()[:, t0 : t0 + SUP],
                        )

                    # s.T = fused_A @ x.T for this supertile: [NR, SUP]
                    s_ps = pss.tile([NR, SUP], F32, tag="sps")
                    for k in range(KC):
                        nc.tensor.matmul(
                            s_ps[:],
                            a_sb[:, k, :],
                            x_sb[:, k, :],
                            start=(k == 0),
                            stop=(k == KC - 1),
                        )
                    sm_sb = sp.tile([NR, SUP], F32R, tag="sm")
                    nc.vector.tensor_mul(
                        sm_sb[:], s_ps[:], m_sb[:, t0 : t0 + SUP]
                    )

                    o_sb = op.tile([P, N_SUB, D_OUT], F32, tag="o")
                    for q in range(N_SUB):
                        ts = q * SUB
                        o_ps = pso.tile([P, D_OUT], F32, tag="ops")
                        if n_inner:
                            for k in range(KC):
                                for n in range(NB):
                                    nsl = slice(n * 512, (n + 1) * 512)
                                    nc.tensor.matmul(
                                        o_ps[:, nsl],
                                        x_sb[:, k, ts : ts + SUB],
                                        w_sb[:, k, nsl],
                                        start=(k == 0),
                                        stop=False,
                                        skip_group_check=True,
                                    )
                            for n in range(NB):
                                nsl = slice(n * 512, (n + 1) * 512)
                                nc.tensor.matmul(
                                    o_ps[:, nsl],
                                    sm_sb[:, ts : ts + SUB],
                                    bt_sb[:, nsl],
                                    start=False,
                                    stop=True,
                                    skip_group_check=True,
                                )
                        else:
                            for n in range(NB):
                                nsl = slice(n * 512, (n + 1) * 512)
                                for k in range(KC):
                                    nc.tensor.matmul(
                                        o_ps[:, nsl],
                                        x_sb[:, k, ts : ts + SUB],
                                        w_sb[:, k, nsl],
                                        start=(k == 0),
                                        stop=False,
                                    )
                                nc.tensor.matmul(
                                    o_ps[:, nsl],
                                    sm_sb[:, ts : ts + SUB],
                                    bt_sb[:, nsl],
                                    start=False,
                                    stop=True,
                                )
                        if split_bias:
                            for n in range(NB):
                                nsl = slice(n * 512, (n + 1) * 512)
                                nc.vector.tensor_add(
                                    o_sb[:, q, nsl], o_ps[:, nsl], b_sb[:, nsl]
                                )
                        else:
                            nc.vector.tensor_add(o_sb[:, q, :], o_ps[:], b_sb[:])
                    nc.scalar.dma_start(out=out_r[s], in_=o_sb[:])

    nc.compile()
    return nc


_NC_CACHE = None


def _get_nc():
    global _NC_CACHE
    if _NC_CACHE is None:
        _NC_CACHE = build_bass()
    return _NC_CACHE


def make_in_maps(x, W, b, lora_A, lora_B, masks):
    x = np.ascontiguousarray(x, dtype=np.float32)
    W = np.ascontiguousarray(W, dtype=np.float32)
    b = np.ascontiguousarray(b, dtype=np.float32)
    lora_A = np.ascontiguousarray(lora_A, dtype=np.float32)
    lora_B = np.ascontiguousarray(lora_B, dtype=np.float32)
    masks = np.ascontiguousarray(masks, dtype=np.float32)

    x_flat = x.reshape(B * T, D_IN)
    A_flat = lora_A.reshape(NR, D_IN)
    B_flat = lora_B.transpose(1, 0, 2).reshape(D_OUT, NR)

    wt = np.ascontiguousarray(W.T)            # [D_IN, D_OUT]
    at = np.ascontiguousarray(A_flat.T)       # [D_IN, NR]
    btr = np.ascontiguousarray(B_flat.T)      # [NR, D_OUT]

    m_full = masks[..., 0].reshape(N_ADAPT, B * T) * np.float32(SCALING)
    m_exp = np.repeat(m_full, R, axis=0)      # [NR, B*T]

    in_maps = []
    for c in range(N_CORES):
        sl = slice(c * TOK, (c + 1) * TOK)
        in_maps.append(
            {
                "xt": np.ascontiguousarray(x_flat[sl].T),
                "wt": wt,
                "at": at,
                "btr": btr,
                "bias": b,
                "msk": np.ascontiguousarray(m_exp[:, sl]),
            }
        )
    return in_maps


def kernel(x, W, b, lora_A, lora_B, masks):
    nc = _get_nc()
    in_maps = make_in_maps(x, W, b, lora_A, lora_B, masks)
    res = run_bass_kernel_spmd(nc, in_maps, core_ids=list(range(N_CORES)))
    out = np.concatenate([r["out"] for r in res.results], axis=0)
    out = out.reshape(B, T, D_OUT)
    return out



# revision 9
# speedup vs baseline: 1.3901x; 1.3901x over previous
"""Routed-LoRA linear layer (moe_routing) on 8 trn2 NeuronCores.

Math (per token t):
  out[t, :] = W @ x[t] + b + 2.0 * sum_n mask[n, t] * (B_n @ (A_n @ x[t]))

Strategy (v3, fused per-adapter weights + fp8 DoubleRow):
  - The routing is one-hot per token, so the whole layer collapses to
        out[t] = W_eff[route(t)] @ x[t] + b,   W_eff_n = W + 2 * B_n @ A_n
    with the four W_eff_n precomputed on the host. Tokens are sorted by
    adapter on the host (gather), so every 128-token matmul subtile uses a
    single W_eff_n; the host scatters the output back to token order.
  - Data-parallel over B*T = 65536 tokens: 8192 tokens per core. Each
    adapter group is padded to a multiple of 128 tokens; group capacities
    are the max over cores so all cores share one SPMD program. The
    subtile->adapter map depends on the routing counts, so the bass
    program is built (and cached) per counts signature.
  - The dense GEMM runs as THREE fp8e4m3 DoubleRow passes accumulated in
    one PSUM group:
        x8 @ W8  +  xlo @ W8  +  (x8/32)_q @ (32*(W_eff - W8))_q
    where x8 = q8(x), xlo = q8(x - x8) (kills the x-quantization error),
    and the third term is a scale-ridden W-residual correction (the W
    residual ~1e-3 sits below e4m3's subnormal floor, so it is scaled up
    32x and the 1/32 rides in a pre-scaled copy of x8). All three terms
    land at the correct absolute scale, so they share one PSUM group.
    Measured max-rel error of this scheme is ~1.5e-3.
  - DoubleRow contracts two 128-row k-tiles per instruction at half the
    per-row cost, so each pass is 4x cheaper than an fp32r pass.
  - Eviction: Activation-engine PSUM->SBUF fp16 copy, then a DVE fp16
    in-place bias add. Output DMA'd as fp16; the host casts to fp32.
"""

import numpy as np
import ml_dtypes

import concourse.bass as bass
from concourse import bacc
import concourse.mybir as mybir
import concourse.tile as tile
from concourse.bass_utils import run_bass_kernel_spmd

N_CORES = 8
B, T = 8, 8192
D_IN = 1024
D_OUT = 1024
N_ADAPT, R = 4, 16
NR = N_ADAPT * R
SCALING = 32.0 / 16.0

TOK = B * T // N_CORES  # 8192 tokens per core
SUP = 512               # tokens per supertile
SUB = 128               # tokens per matmul M-tile
N_SUB = SUP // SUB      # 4
P = 128
KC = D_IN // P          # 8 contraction chunks
NB = D_OUT // 512       # 2 PSUM-bank column halves
NREP = 3                # x8, xlo, x8s
WSCALE = 32.0

F32 = mybir.dt.float32
F16 = mybir.dt.float16
F8 = mybir.dt.float8e4
DR = mybir.MatmulPerfMode.DoubleRow

E4 = ml_dtypes.float8_e4m3


def build_bass(sub_adapters):
    """sub_adapters: tuple of adapter ids, one per 128-token subtile;
    length must be a multiple of N_SUB."""
    n_sub = len(sub_adapters)
    assert n_sub % N_SUB == 0
    n_sup = n_sub // N_SUB
    cap = n_sub * SUB

    nc = bacc.Bacc(
        "TRN2", target_bir_lowering=False, debug=False, num_devices=N_CORES
    )

    xcat_d = nc.dram_tensor(
        "xcat", [n_sup * P, NREP * KC * SUP], F8, kind="ExternalInput"
    )
    # wcat rows: ((g*2 + grp)*KC + kc, p); cols: n.  grp 0 = W8, 1 = W32
    wcat_d = nc.dram_tensor(
        "wcat", [N_ADAPT * 2 * KC * P, D_OUT], F8, kind="ExternalInput"
    )
    bias_d = nc.dram_tensor("bias", [D_OUT], F16, kind="ExternalInput")
    out_d = nc.dram_tensor("out", [cap, D_OUT], F16, kind="ExternalOutput")

    xcat_r = xcat_d.ap().rearrange("(s p) (c t) -> s p c t", p=P, t=SUP)
    wcat_r = wcat_d.ap().rearrange("(c p) n -> p c n", p=P)
    out_r = out_d.ap().rearrange("(s q p) n -> s p q n", q=N_SUB, p=P)
    bias_bcast = bass.AP(tensor=bias_d, offset=0, ap=[[0, P], [1, D_OUT]])

    # distinct adapters in first-use order, for W preload scheduling
    first_use = []
    for a in sub_adapters:
        if a not in first_use:
            first_use.append(a)
    for a in range(N_ADAPT):
        if a not in first_use:
            first_use.append(a)
    a0 = first_use[0]

    with tile.TileContext(nc) as tc:
        with (
            tc.tile_pool(name="const", bufs=1) as const,
            tc.tile_pool(name="xp", bufs=3) as xp,
            tc.tile_pool(name="op", bufs=2) as op,
            tc.tile_pool(name="pso", bufs=6, space="PSUM") as pso,
        ):
            w_sb = const.tile([P, N_ADAPT * 2 * KC, D_OUT], F8)
            b_sb = const.tile([P, D_OUT], F16)

            def wslice(g, grp, c):
                return (g * 2 + grp) * KC + c

            # Startup-critical loads on the sync queue in exact first-use
            # order (single queue => FIFO through the DGE => DMA device
            # drains in this order): x8(s0) gates pass 0 with W8[a0],
            # xlo(s0) pass 1, W32[a0] + x8s(s0) pass 2.
            x_sb0 = xp.tile([P, NREP * KC, SUP], F8, tag="x")
            nc.sync.dma_start(out=x_sb0[:, :KC, :], in_=xcat_r[0][:, :KC, :])
            for c in range(KC):
                nc.sync.dma_start(
                    out=w_sb[:, wslice(a0, 0, c), :],
                    in_=wcat_r[:, wslice(a0, 0, c), :],
                )
            nc.sync.dma_start(
                out=x_sb0[:, KC : 2 * KC, :], in_=xcat_r[0][:, KC : 2 * KC, :]
            )
            for c in range(KC):
                nc.sync.dma_start(
                    out=w_sb[:, wslice(a0, 1, c), :],
                    in_=wcat_r[:, wslice(a0, 1, c), :],
                )
            nc.sync.dma_start(
                out=x_sb0[:, 2 * KC :, :], in_=xcat_r[0][:, 2 * KC :, :]
            )
            nc.gpsimd.dma_start(out=b_sb[:], in_=bias_bcast)
            # Remaining adapters' weights trickle in on the scalar queue.
            for g in first_use[1:]:
                for grp in range(2):
                    for c in range(KC):
                        nc.scalar.dma_start(
                            out=w_sb[:, wslice(g, grp, c), :],
                            in_=wcat_r[:, wslice(g, grp, c), :],
                        )

            for s in range(n_sup):
                if s == 0:
                    x_sb = x_sb0
                else:
                    x_sb = xp.tile([P, NREP * KC, SUP], F8, tag="x")
                    nc.sync.dma_start(
                        out=x_sb[:, : 2 * KC, :], in_=xcat_r[s][:, : 2 * KC, :]
                    )
                    nc.sync.dma_start(
                        out=x_sb[:, 2 * KC :, :], in_=xcat_r[s][:, 2 * KC :, :]
                    )

                o_sb = op.tile([P, N_SUB, D_OUT], F16, tag="o")
                for q in range(N_SUB):
                    ts = q * SUB
                    g = sub_adapters[s * N_SUB + q]
                    for n in range(NB):
                        nsl = slice(n * 512, (n + 1) * 512)
                        o_ps = pso.tile([P, 512], F32, tag="ops")
                        for rep, grp in ((0, 0), (1, 0), (2, 1)):
                            xo = rep * KC
                            wo = wslice(g, grp, 0)
                            for j in range(KC // 2):
                                nc.tensor.matmul(
                                    o_ps[:],
                                    x_sb[
                                        :, xo + 2 * j : xo + 2 * j + 2, ts : ts + SUB
                                    ],
                                    w_sb[:, wo + 2 * j : wo + 2 * j + 2, nsl],
                                    start=(rep == 0 and j == 0),
                                    stop=(rep == 2 and j == KC // 2 - 1),
                                    perf_mode=DR,
                                )
                        # PSUM->SBUF fp16 on Act, then fp16 bias add on DVE
                        nc.scalar.copy(o_sb[:, q, nsl], o_ps[:])
                        nc.vector.tensor_add(
                            o_sb[:, q, nsl], o_sb[:, q, nsl], b_sb[:, nsl]
                        )
                    if q == 1:
                        nc.gpsimd.dma_start(
                            out=out_r[s][:, 0:2, :], in_=o_sb[:, 0:2, :]
                        )
                nc.gpsimd.dma_start(out=out_r[s][:, 2:4, :], in_=o_sb[:, 2:4, :])

    nc.compile()
    return nc


_NC_CACHE = {}
_NC_LAST = None


def _get_nc(sub_adapters=None):
    global _NC_LAST
    if sub_adapters is None:
        return _NC_LAST
    key = tuple(sub_adapters)
    if key not in _NC_CACHE:
        _NC_CACHE[key] = build_bass(key)
    _NC_LAST = _NC_CACHE[key]
    return _NC_LAST


def _q8(a):
    return np.asarray(a, dtype=np.float32).astype(E4)


def kernel(x, W, b, lora_A, lora_B, masks):
    x = np.ascontiguousarray(x, dtype=np.float32)
    W = np.ascontiguousarray(W, dtype=np.float32)
    b = np.ascontiguousarray(b, dtype=np.float32)
    lora_A = np.ascontiguousarray(lora_A, dtype=np.float32)
    lora_B = np.ascontiguousarray(lora_B, dtype=np.float32)
    masks = np.ascontiguousarray(masks, dtype=np.float32)

    x_flat = x.reshape(B * T, D_IN)
    route = np.argmax(masks[..., 0].reshape(N_ADAPT, B * T), axis=0)

    # --- per-core routing counts -> shared subtile layout ---
    counts = np.zeros((N_CORES, N_ADAPT), dtype=np.int64)
    orders = []
    for c in range(N_CORES):
        r = route[c * TOK : (c + 1) * TOK]
        orders.append(np.argsort(r, kind="stable"))
        counts[c] = np.bincount(r, minlength=N_ADAPT)
    n_sub_g = np.maximum(
        (counts.max(axis=0) + SUB - 1) // SUB, 1
    )  # subtiles per adapter group
    n_sub = int(n_sub_g.sum())
    n_sub = (n_sub + N_SUB - 1) // N_SUB * N_SUB  # round up to full supertiles
    base_sub = np.zeros(N_ADAPT, dtype=np.int64)
    base_sub[1:] = np.cumsum(n_sub_g)[:-1]
    cap = n_sub * SUB
    n_sup = n_sub // N_SUB

    sub_adapters = []
    for g in range(N_ADAPT):
        sub_adapters += [g] * int(n_sub_g[g])
    sub_adapters += [N_ADAPT - 1] * (n_sub - len(sub_adapters))
    nc = _get_nc(tuple(sub_adapters))

    # --- fused per-adapter weights, split into fp8 hi + scaled residual ---
    A_flat = lora_A.reshape(NR, D_IN)
    B_flat = lora_B.transpose(1, 0, 2).reshape(D_OUT, NR)
    w8s, w32s = [], []
    for g in range(N_ADAPT):
        W_eff = W + np.float32(SCALING) * (
            B_flat[:, g * R : (g + 1) * R] @ A_flat[g * R : (g + 1) * R, :]
        )
        wt = np.ascontiguousarray(W_eff.T)
        W8 = _q8(wt)
        W32 = _q8(WSCALE * (wt - W8.astype(np.float32)))
        w8s.append(W8.reshape(KC, P, D_OUT))
        w32s.append(W32.reshape(KC, P, D_OUT))
    wcat = np.stack(
        [np.concatenate([w8s[g], w32s[g]], axis=0) for g in range(N_ADAPT)],
        axis=0,
    ).reshape(N_ADAPT * 2 * KC * P, D_OUT)

    b16 = b.astype(np.float16)

    # --- per-core token gather + fp8 reps ---
    in_maps = []
    dsts = []
    for c in range(N_CORES):
        sl = slice(c * TOK, (c + 1) * TOK)
        r = route[sl]
        order = orders[c]
        dst = np.zeros(TOK, dtype=np.int64)
        pos = 0
        for g in range(N_ADAPT):
            cnt = int(counts[c, g])
            dst[order[pos : pos + cnt]] = base_sub[g] * SUB + np.arange(cnt)
            pos += cnt
        dsts.append(dst)

        x_perm = np.zeros((cap, D_IN), dtype=np.float32)
        x_perm[dst] = x_flat[sl]

        x8 = _q8(x_perm)
        x8f = x8.astype(np.float32)
        xlo = _q8(x_perm - x8f)
        x8s = _q8(x8f / WSCALE)
        reps = []
        for arr in (x8, xlo, x8s):
            shard = arr.reshape(n_sup, SUP, KC, P)
            reps.append(shard.transpose(0, 3, 2, 1))  # [s, p, k, t]
        xcat = np.stack(reps, axis=2)  # [s, p, rep, k, t]
        xcat = np.ascontiguousarray(xcat).reshape(n_sup * P, NREP * KC * SUP)
        in_maps.append({"xcat": xcat, "wcat": wcat, "bias": b16})

    res = run_bass_kernel_spmd(nc, in_maps, core_ids=list(range(N_CORES)))
    out = np.empty((B * T, D_OUT), dtype=np.float32)
    for c in range(N_CORES):
        o = np.asarray(res.results[c]["out"], dtype=np.float32)
        out[c * TOK : (c + 1) * TOK] = o[dsts[c]]
    return out.reshape(B, T, D_OUT)


# revision 21
# speedup vs baseline: 1.5343x; 1.1037x over previous
"""Routed-LoRA linear layer (moe_routing) on 8 trn2 NeuronCores.

Math (per token t):
  out[t, :] = W @ x[t] + b + 2.0 * sum_n mask[n, t] * (B_n @ (A_n @ x[t]))

Strategy (v3, fused per-adapter weights + fp8 DoubleRow):
  - The routing is one-hot per token, so the whole layer collapses to
        out[t] = W_eff[route(t)] @ x[t] + b,   W_eff_n = W + 2 * B_n @ A_n
    with the four W_eff_n precomputed on the host. Tokens are sorted by
    adapter on the host (gather), so every 128-token matmul subtile uses a
    single W_eff_n; the host scatters the output back to token order.
  - Data-parallel over B*T = 65536 tokens: 8192 tokens per core. Each
    adapter group is padded to a multiple of 128 tokens; group capacities
    are the max over cores so all cores share one SPMD program. The
    subtile->adapter map depends on the routing counts, so the bass
    program is built (and cached) per counts signature.
  - The dense GEMM runs as THREE fp8e4m3 DoubleRow passes accumulated in
    one PSUM group:
        x8 @ W8  +  xlo @ W8  +  (x8/32)_q @ (32*(W_eff - W8))_q
    where x8 = q8(x), xlo = q8(x - x8) (kills the x-quantization error),
    and the third term is a scale-ridden W-residual correction (the W
    residual ~1e-3 sits below e4m3's subnormal floor, so it is scaled up
    32x and the 1/32 rides in a pre-scaled copy of x8). All three terms
    land at the correct absolute scale, so they share one PSUM group.
    Measured max-rel error of this scheme is ~1.5e-3.
  - DoubleRow contracts two 128-row k-tiles per instruction at half the
    per-row cost, so each pass is 4x cheaper than an fp32r pass.
  - Eviction: Activation-engine PSUM->SBUF fp16 copy, then a DVE fp16
    in-place bias add. Output DMA'd as fp16; the host casts to fp32.
"""

import numpy as np
import ml_dtypes

import concourse.bass as bass
from concourse import bacc
import concourse.mybir as mybir
import concourse.tile as tile
from concourse.bass_utils import run_bass_kernel_spmd

N_CORES = 8
B, T = 8, 8192
D_IN = 1024
D_OUT = 1024
N_ADAPT, R = 4, 16
NR = N_ADAPT * R
SCALING = 32.0 / 16.0

TOK = B * T // N_CORES  # 8192 tokens per core
SUP = 512               # tokens per supertile
SUB = 128               # tokens per matmul M-tile
N_SUB = SUP // SUB      # 4
P = 128
KC = D_IN // P          # 8 contraction chunks
NB = D_OUT // 512       # 2 PSUM-bank column halves
NREP = 3                # x8, xlo, x8s
WSCALE = 32.0

F32 = mybir.dt.float32
F16 = mybir.dt.float16
F8 = mybir.dt.float8e4
DR = mybir.MatmulPerfMode.DoubleRow

E4 = ml_dtypes.float8_e4m3


def build_bass(sub_adapters):
    """sub_adapters: tuple of adapter ids, one per 128-token subtile;
    length must be even (the last supertile may hold 2 subtiles)."""
    n_sub = len(sub_adapters)
    assert n_sub % 2 == 0
    n_sup = (n_sub + N_SUB - 1) // N_SUB
    rem_last = n_sub - N_SUB * (n_sup - 1)
    cap = n_sub * SUB

    nc = bacc.Bacc(
        "TRN2", target_bir_lowering=False, debug=False, num_devices=N_CORES
    )

    xcat_d = nc.dram_tensor(
        "xcat", [n_sup * P, NREP * KC * SUP], F8, kind="ExternalInput"
    )
    # wcat rows: ((g*2 + grp)*KC + kc, p); cols: n.  grp 0 = W8, 1 = W32
    wcat_d = nc.dram_tensor(
        "wcat", [N_ADAPT * 2 * KC * P, D_OUT], F8, kind="ExternalInput"
    )
    bias_d = nc.dram_tensor("bias", [D_OUT], F16, kind="ExternalInput")
    out_d = nc.dram_tensor("out", [cap, D_OUT], F16, kind="ExternalOutput")

    xcat_r = xcat_d.ap().rearrange("(s p) (c t) -> s p c t", p=P, t=SUP)
    wcat_r = wcat_d.ap().rearrange("(c p) n -> p c n", p=P)
    out_half = out_d.ap().rearrange("(h q p) n -> h p q n", q=2, p=P)
    out_sub = out_d.ap().rearrange("(qq p) n -> qq p n", p=P)
    bias_bcast = bass.AP(tensor=bias_d, offset=0, ap=[[0, P], [1, D_OUT]])

    # distinct adapters in first-use order, for W preload scheduling
    first_use = []
    for a in sub_adapters:
        if a not in first_use:
            first_use.append(a)
    for a in range(N_ADAPT):
        if a not in first_use:
            first_use.append(a)
    a0 = first_use[0]

    with tile.TileContext(nc) as tc:
        with (
            tc.tile_pool(name="const", bufs=1) as const,
            tc.tile_pool(name="xp", bufs=4) as xp,
            tc.tile_pool(name="op", bufs=2) as op,
            tc.tile_pool(name="pso", bufs=8, space="PSUM") as pso,
        ):
            w_sb = const.tile([P, N_ADAPT * 2 * KC, D_OUT], F8)
            b_sb = const.tile([P, D_OUT], F16)

            def wslice(g, grp, c):
                return (g * 2 + grp) * KC + c

            # Startup-critical loads on the sync queue in exact first-use
            # order (single queue => FIFO through the DGE => DMA device
            # drains in this order): x8(s0) gates pass 0 with W8[a0],
            # xlo(s0) pass 1, W32[a0] + x8s(s0) pass 2.
            x_sb0 = xp.tile([P, NREP * KC, SUP], F8, tag="x")
            nc.sync.dma_start(out=x_sb0[:, :KC, :], in_=xcat_r[0][:, :KC, :])
            for c in range(KC):
                nc.sync.dma_start(
                    out=w_sb[:, wslice(a0, 0, c), :],
                    in_=wcat_r[:, wslice(a0, 0, c), :],
                )
            nc.sync.dma_start(
                out=x_sb0[:, KC : 2 * KC, :], in_=xcat_r[0][:, KC : 2 * KC, :]
            )
            for c in range(KC):
                nc.sync.dma_start(
                    out=w_sb[:, wslice(a0, 1, c), :],
                    in_=wcat_r[:, wslice(a0, 1, c), :],
                )
            nc.sync.dma_start(
                out=x_sb0[:, 2 * KC :, :], in_=xcat_r[0][:, 2 * KC :, :]
            )
            nc.gpsimd.dma_start(out=b_sb[:], in_=bias_bcast)
            # Remaining adapters' W chunks trickle in 4-per-supertile (on
            # the gpsimd queue) so they never displace the x streams on
            # the DMA device during the first supertiles.
            pending_w = [
                (g, grp, c)
                for g in first_use[1:]
                for grp in range(2)
                for c in range(KC)
            ]

            for s in range(n_sup):
                if s == 0:
                    x_sb = x_sb0
                else:
                    x_sb = xp.tile([P, NREP * KC, SUP], F8, tag="x")
                    nc.sync.dma_start(
                        out=x_sb[:, : 2 * KC, :], in_=xcat_r[s][:, : 2 * KC, :]
                    )
                    nc.sync.dma_start(
                        out=x_sb[:, 2 * KC :, :], in_=xcat_r[s][:, 2 * KC :, :]
                    )
                # scalar queue: decoupled from the out-DMAs (which wait on
                # evictions and would stall a FIFO queue behind them)
                for g, grp, c in pending_w[6 * s : 6 * s + 6]:
                    nc.scalar.dma_start(
                        out=w_sb[:, wslice(g, grp, c), :],
                        in_=wcat_r[:, wslice(g, grp, c), :],
                    )

                o_sb = op.tile([P, N_SUB, D_OUT], F16, tag="o")
                n_q = rem_last if s == n_sup - 1 else N_SUB
                for q in range(n_q):
                    ts = q * SUB
                    g = sub_adapters[s * N_SUB + q]
                    for n in range(NB):
                        nsl = slice(n * 512, (n + 1) * 512)
                        o_ps = pso.tile([P, 512], F32, tag="ops")
                        for rep, grp in ((0, 0), (1, 0), (2, 1)):
                            xo = rep * KC
                            wo = wslice(g, grp, 0)
                            for j in range(KC // 2):
                                nc.tensor.matmul(
                                    o_ps[:],
                                    x_sb[
                                        :, xo + 2 * j : xo + 2 * j + 2, ts : ts + SUB
                                    ],
                                    w_sb[:, wo + 2 * j : wo + 2 * j + 2, nsl],
                                    start=(rep == 0 and j == 0),
                                    stop=(rep == 2 and j == KC // 2 - 1),
                                    perf_mode=DR,
                                )
                        # PSUM->SBUF fp16 on Act, then fp16 bias add on DVE
                        nc.scalar.copy(o_sb[:, q, nsl], o_ps[:])
                        nc.vector.tensor_add(
                            o_sb[:, q, nsl], o_sb[:, q, nsl], b_sb[:, nsl]
                        )
                    if s == n_sup - 1:
                        # last supertile: per-subtile out DMA to shrink tail
                        nc.gpsimd.dma_start(
                            out=out_sub[s * N_SUB + q], in_=o_sb[:, q, :]
                        )
                    elif q == 1:
                        nc.gpsimd.dma_start(
                            out=out_half[2 * s], in_=o_sb[:, 0:2, :]
                        )
                if s != n_sup - 1:
                    nc.gpsimd.dma_start(
                        out=out_half[2 * s + 1], in_=o_sb[:, 2:4, :]
                    )

    nc.compile()
    return nc


_NC_CACHE = {}
_NC_LAST = None


def _get_nc(sub_adapters=None):
    global _NC_LAST
    if sub_adapters is None:
        return _NC_LAST
    key = tuple(sub_adapters)
    if key not in _NC_CACHE:
        _NC_CACHE[key] = build_bass(key)
    _NC_LAST = _NC_CACHE[key]
    return _NC_LAST


def _q8(a):
    return np.asarray(a, dtype=np.float32).astype(E4)


def kernel(x, W, b, lora_A, lora_B, masks):
    x = np.ascontiguousarray(x, dtype=np.float32)
    W = np.ascontiguousarray(W, dtype=np.float32)
    b = np.ascontiguousarray(b, dtype=np.float32)
    lora_A = np.ascontiguousarray(lora_A, dtype=np.float32)
    lora_B = np.ascontiguousarray(lora_B, dtype=np.float32)
    masks = np.ascontiguousarray(masks, dtype=np.float32)

    x_flat = x.reshape(B * T, D_IN)
    route = np.argmax(masks[..., 0].reshape(N_ADAPT, B * T), axis=0)

    # --- global balancing: tokens are dealt to cores so every core gets
    # the same per-adapter capacity (the host gathers/scatters globally
    # anyway), minimizing subtile padding across the SPMD program. ---
    G = np.bincount(route, minlength=N_ADAPT)
    n_sub_g = np.maximum((G + N_CORES * SUB - 1) // (N_CORES * SUB), 1)
    n_sub = int(n_sub_g.sum())
    if n_sub % 2:  # keep supertile halves intact
        n_sub_g[int(np.argmin(n_sub_g))] += 1
        n_sub += 1
    base_sub = np.zeros(N_ADAPT, dtype=np.int64)
    base_sub[1:] = np.cumsum(n_sub_g)[:-1]
    cap = n_sub * SUB  # tokens per core (incl. padding)
    n_sup = (n_sub + N_SUB - 1) // N_SUB

    sub_adapters = []
    for g in range(N_ADAPT):
        sub_adapters += [g] * int(n_sub_g[g])
    nc = _get_nc(tuple(sub_adapters))

    # token assignment: adapter-g tokens split into 8 near-equal chunks
    src_tokens = [[] for _ in range(N_CORES)]  # global token idx per core
    dst_slots = [[] for _ in range(N_CORES)]   # matching slot per core
    for g in range(N_ADAPT):
        idx_g = np.nonzero(route == g)[0]
        base = len(idx_g) // N_CORES
        extra = len(idx_g) % N_CORES
        off = 0
        for c in range(N_CORES):
            cnt = base + (1 if c < extra else 0)
            src_tokens[c].append(idx_g[off : off + cnt])
            dst_slots[c].append(base_sub[g] * SUB + np.arange(cnt))
            off += cnt

    # --- fused per-adapter weights, split into fp8 hi + scaled residual ---
    A_flat = lora_A.reshape(NR, D_IN)
    B_flat = lora_B.transpose(1, 0, 2).reshape(D_OUT, NR)
    w8s, w32s = [], []
    for g in range(N_ADAPT):
        W_eff = W + np.float32(SCALING) * (
            B_flat[:, g * R : (g + 1) * R] @ A_flat[g * R : (g + 1) * R, :]
        )
        wt = np.ascontiguousarray(W_eff.T)
        W8 = _q8(wt)
        W32 = _q8(WSCALE * (wt - W8.astype(np.float32)))
        w8s.append(W8.reshape(KC, P, D_OUT))
        w32s.append(W32.reshape(KC, P, D_OUT))
    wcat = np.stack(
        [np.concatenate([w8s[g], w32s[g]], axis=0) for g in range(N_ADAPT)],
        axis=0,
    ).reshape(N_ADAPT * 2 * KC * P, D_OUT)

    b16 = b.astype(np.float16)

    # --- per-core token gather + fp8 reps ---
    sup_cap = n_sup * SUP  # x dram capacity (last supertile zero-padded)
    in_maps = []
    srcs, dsts = [], []
    for c in range(N_CORES):
        src = np.concatenate(src_tokens[c])
        dst = np.concatenate(dst_slots[c]).astype(np.int64)
        srcs.append(src)
        dsts.append(dst)

        x_perm = np.zeros((sup_cap, D_IN), dtype=np.float32)
        x_perm[dst] = x_flat[src]

        x8 = _q8(x_perm)
        x8f = x8.astype(np.float32)
        xlo = _q8(x_perm - x8f)
        x8s = _q8(x8f / WSCALE)
        reps = []
        for arr in (x8, xlo, x8s):
            shard = arr.reshape(n_sup, SUP, KC, P)
            reps.append(shard.transpose(0, 3, 2, 1))  # [s, p, k, t]
        xcat = np.stack(reps, axis=2)  # [s, p, rep, k, t]
        xcat = np.ascontiguousarray(xcat).reshape(n_sup * P, NREP * KC * SUP)
        in_maps.append({"xcat": xcat, "wcat": wcat, "bias": b16})

    res = run_bass_kernel_spmd(nc, in_maps, core_ids=list(range(N_CORES)))
    out = np.empty((B * T, D_OUT), dtype=np.float32)
    for c in range(N_CORES):
        o = np.asarray(res.results[c]["out"], dtype=np.float32)
        out[srcs[c]] = o[dsts[c]]
    return out.reshape(B, T, D_OUT)


# revision 29
# speedup vs baseline: 1.7082x; 1.1134x over previous
"""Routed-LoRA linear layer (moe_routing) on 8 trn2 NeuronCores.

Math (per token t):
  out[t, :] = W @ x[t] + b + 2.0 * sum_n mask[n, t] * (B_n @ (A_n @ x[t]))

Strategy (v3, fused per-adapter weights + fp8 DoubleRow):
  - The routing is one-hot per token, so the whole layer collapses to
        out[t] = W_eff[route(t)] @ x[t] + b,   W_eff_n = W + 2 * B_n @ A_n
    with the four W_eff_n precomputed on the host. Tokens are sorted by
    adapter on the host (gather), so every 128-token matmul subtile uses a
    single W_eff_n; the host scatters the output back to token order.
  - Data-parallel over B*T = 65536 tokens: 8192 tokens per core. Each
    adapter group is padded to a multiple of 128 tokens; group capacities
    are the max over cores so all cores share one SPMD program. The
    subtile->adapter map depends on the routing counts, so the bass
    program is built (and cached) per counts signature.
  - The dense GEMM runs as THREE fp8e4m3 DoubleRow passes accumulated in
    one PSUM group:
        x8 @ W8  +  xlo @ W8  +  (x8/32)_q @ (32*(W_eff - W8))_q
    where x8 = q8(x), xlo = q8(x - x8) (kills the x-quantization error),
    and the third term is a scale-ridden W-residual correction (the W
    residual ~1e-3 sits below e4m3's subnormal floor, so it is scaled up
    32x and the 1/32 rides in a pre-scaled copy of x8). All three terms
    land at the correct absolute scale, so they share one PSUM group.
    Measured max-rel error of this scheme is ~1.5e-3.
  - DoubleRow contracts two 128-row k-tiles per instruction at half the
    per-row cost, so each pass is 4x cheaper than an fp32r pass.
  - Eviction: Activation-engine PSUM->SBUF fp16 copy, then a DVE fp16
    in-place bias add. Output DMA'd as fp16; the host casts to fp32.
"""

import numpy as np
import ml_dtypes

import concourse.bass as bass
from concourse import bacc
import concourse.mybir as mybir
import concourse.tile as tile
from concourse.bass_utils import run_bass_kernel_spmd

N_CORES = 8
B, T = 8, 8192
D_IN = 1024
D_OUT = 1024
N_ADAPT, R = 4, 16
NR = N_ADAPT * R
SCALING = 32.0 / 16.0

TOK = B * T // N_CORES  # 8192 tokens per core
SUP = 512               # tokens per supertile
SUB = 128               # tokens per matmul M-tile
N_SUB = SUP // SUB      # 4
P = 128
KC = D_IN // P          # 8 contraction chunks
NB = D_OUT // 512       # 2 PSUM-bank column halves
NREP = 3                # x8, xlo, x8s
WSCALE = 32.0

F32 = mybir.dt.float32
F16 = mybir.dt.float16
F8 = mybir.dt.float8e4
DR = mybir.MatmulPerfMode.DoubleRow

E4 = ml_dtypes.float8_e4m3


def build_bass(sub_adapters):
    """sub_adapters: tuple of adapter ids, one per 128-token subtile;
    length must be even (the last supertile may hold 2 subtiles)."""
    n_sub = len(sub_adapters)
    assert n_sub % 2 == 0
    n_sup = (n_sub + N_SUB - 1) // N_SUB
    rem_last = n_sub - N_SUB * (n_sup - 1)
    cap = n_sub * SUB

    nc = bacc.Bacc(
        "TRN2", target_bir_lowering=False, debug=False, num_devices=N_CORES
    )

    xcat_d = nc.dram_tensor(
        "xcat", [n_sup * P, NREP * KC * SUP], F8, kind="ExternalInput"
    )
    # wcat rows: ((g*2 + grp)*KC + kc, p); cols: n.  grp 0 = W8, 1 = W32
    wcat_d = nc.dram_tensor(
        "wcat", [N_ADAPT * 2 * KC * P, D_OUT], F8, kind="ExternalInput"
    )
    bias_d = nc.dram_tensor("bias", [D_OUT], F16, kind="ExternalInput")
    out_d = nc.dram_tensor("out", [cap, D_OUT], F16, kind="ExternalOutput")

    xcat_r = xcat_d.ap().rearrange("(s p) (c t) -> s p c t", p=P, t=SUP)
    wcat_r = wcat_d.ap().rearrange("(c p) n -> p c n", p=P)
    out_half = out_d.ap().rearrange("(h q p) n -> h p q n", q=2, p=P)
    out_sub = out_d.ap().rearrange("(qq p) n -> qq p n", p=P)
    bias_bcast = bass.AP(tensor=bias_d, offset=0, ap=[[0, P], [1, D_OUT]])

    # distinct adapters in first-use order, for W preload scheduling
    first_use = []
    for a in sub_adapters:
        if a not in first_use:
            first_use.append(a)
    for a in range(N_ADAPT):
        if a not in first_use:
            first_use.append(a)
    a0 = first_use[0]

    with tile.TileContext(nc) as tc:
        with (
            tc.tile_pool(name="const", bufs=1) as const,
            tc.tile_pool(name="xp", bufs=4) as xp,
            tc.tile_pool(name="op", bufs=2) as op,
            tc.tile_pool(name="pso", bufs=8, space="PSUM") as pso,
        ):
            w_sb = const.tile([P, N_ADAPT * 2 * KC, D_OUT], F8)
            b_sb = const.tile([P, D_OUT], F16)

            def wslice(g, grp, c):
                return (g * 2 + grp) * KC + c

            # Startup-critical loads on the sync queue in exact first-use
            # order (single queue => FIFO through the DGE => DMA device
            # drains in this order): x8(s0) gates pass 0 with W8[a0],
            # xlo(s0) pass 1, W32[a0] + x8s(s0) pass 2.
            # PE p-state pre-warm: ~12 dummy matmuls on a zeroed scratch
            # row keep the tensor engine continuously busy from t~0 so the
            # ramp (0.65 -> 2.4 GHz after 3us of continuous execution) is
            # paid during the startup DMA wait instead of on real work.
            dm = const.tile([1, 640], F8)
            nc.vector.memset(dm[:], 0.0)
            d_ps = pso.tile([P, 512], F32, tag="ops", name="warm")
            for _ in range(12):
                nc.tensor.matmul(
                    d_ps[:],
                    dm[0:1, 0:P],
                    dm[0:1, P : P + 512],
                    start=True,
                    stop=True,
                    skip_group_check=True,
                )

            x_sb0 = xp.tile([P, NREP * KC, SUP], F8, tag="x")
            nc.sync.dma_start(
                out=x_sb0[:, 0:2, :], in_=xcat_r[0][:, 0:2, :]
            )
            nc.sync.dma_start(
                out=x_sb0[:, 2:KC, :], in_=xcat_r[0][:, 2:KC, :]
            )
            for c in range(KC):
                nc.sync.dma_start(
                    out=w_sb[:, wslice(a0, 0, c), :],
                    in_=wcat_r[:, wslice(a0, 0, c), :],
                )
            nc.sync.dma_start(
                out=x_sb0[:, KC : 2 * KC, :], in_=xcat_r[0][:, KC : 2 * KC, :]
            )
            for c in range(KC):
                nc.sync.dma_start(
                    out=w_sb[:, wslice(a0, 1, c), :],
                    in_=wcat_r[:, wslice(a0, 1, c), :],
                )
            nc.sync.dma_start(
                out=x_sb0[:, 2 * KC :, :], in_=xcat_r[0][:, 2 * KC :, :]
            )
            nc.gpsimd.dma_start(out=b_sb[:], in_=bias_bcast)
            # Remaining adapters' W chunks trickle in 4-per-supertile (on
            # the gpsimd queue) so they never displace the x streams on
            # the DMA device during the first supertiles.
            pending_w = [
                (g, grp, c)
                for g in first_use[1:]
                for grp in range(2)
                for c in range(KC)
            ]

            for s in range(n_sup):
                if s == 0:
                    x_sb = x_sb0
                else:
                    x_sb = xp.tile([P, NREP * KC, SUP], F8, tag="x")
                    nc.sync.dma_start(
                        out=x_sb[:, : 2 * KC, :], in_=xcat_r[s][:, : 2 * KC, :]
                    )
                    nc.sync.dma_start(
                        out=x_sb[:, 2 * KC :, :], in_=xcat_r[s][:, 2 * KC :, :]
                    )
                # sync queue, right after this supertile's x loads: the xp
                # pool's WAR dependency paces the FIFO queue at one W batch
                # per supertile without depending on eviction timing.
                for g, grp, c in ([] if s == 0 else pending_w[8 * (s - 1) : 8 * s]):
                    nc.sync.dma_start(
                        out=w_sb[:, wslice(g, grp, c), :],
                        in_=wcat_r[:, wslice(g, grp, c), :],
                    )

                o_sb = op.tile([P, N_SUB, D_OUT], F16, tag="o")
                n_q = rem_last if s == n_sup - 1 else N_SUB

                def emit_group(o_ps, q, nsl, rep_grp):
                    ts = q * SUB
                    g = sub_adapters[s * N_SUB + q]
                    for rep, grp in rep_grp:
                        xo = rep * KC
                        wo = wslice(g, grp, 0)
                        for j in range(KC // 2):
                            nc.tensor.matmul(
                                o_ps[:],
                                x_sb[:, xo + 2 * j : xo + 2 * j + 2, ts : ts + SUB],
                                w_sb[:, wo + 2 * j : wo + 2 * j + 2, nsl],
                                start=(rep == 0 and j == 0),
                                stop=(rep == 2 and j == KC // 2 - 1),
                                perf_mode=DR,
                                skip_group_check=(s == 0),
                            )

                def evict(o_ps, q, nsl, last_sup):
                    # PSUM->SBUF fp16 on Act, then fp16 bias add on DVE;
                    # on the last supertile split whole groups between DVE
                    # (single fused add) and the Act chain so the serial
                    # eviction tail runs on two engines.
                    if last_sup and (q + nsl.start // 512) % 2 == 0:
                        nc.vector.tensor_add(o_sb[:, q, nsl], o_ps[:], b_sb[:, nsl])
                    else:
                        nc.scalar.copy(o_sb[:, q, nsl], o_ps[:])
                        nc.vector.tensor_add(
                            o_sb[:, q, nsl], o_sb[:, q, nsl], b_sb[:, nsl]
                        )

                if s == 0:
                    # pass-major across all 8 PSUM banks: the first pass
                    # only needs x8+W8, which land first, so the PE ramps
                    # while xlo/W32/x8s are still in flight.
                    o_pss = [
                        pso.tile([P, 512], F32, tag="ops", name=f"ops0_{i}")
                        for i in range(8)
                    ]
                    for rep_grp in ((0, 0), (1, 0), (2, 1)):
                        for q in range(N_SUB):
                            for n in range(NB):
                                nsl = slice(n * 512, (n + 1) * 512)
                                emit_group(o_pss[q * NB + n], q, nsl, [rep_grp])
                    for q in range(N_SUB):
                        for n in range(NB):
                            nsl = slice(n * 512, (n + 1) * 512)
                            evict(o_pss[q * NB + n], q, nsl, False)
                    nc.gpsimd.dma_start(out=out_half[0], in_=o_sb[:, 0:2, :])
                    nc.gpsimd.dma_start(out=out_half[1], in_=o_sb[:, 2:4, :])
                else:
                    for q in range(n_q):
                        for n in range(NB):
                            nsl = slice(n * 512, (n + 1) * 512)
                            o_ps = pso.tile([P, 512], F32, tag="ops")
                            emit_group(
                                o_ps, q, nsl, [(0, 0), (1, 0), (2, 1)]
                            )
                            evict(o_ps, q, nsl, s == n_sup - 1)
                        if s == n_sup - 1:
                            # last supertile: per-subtile out DMA (tail)
                            nc.gpsimd.dma_start(
                                out=out_sub[s * N_SUB + q], in_=o_sb[:, q, :]
                            )
                        elif q == 1:
                            nc.gpsimd.dma_start(
                                out=out_half[2 * s], in_=o_sb[:, 0:2, :]
                            )
                    if s != n_sup - 1:
                        nc.gpsimd.dma_start(
                            out=out_half[2 * s + 1], in_=o_sb[:, 2:4, :]
                        )

    nc.compile()
    return nc


_NC_CACHE = {}
_NC_LAST = None


def _get_nc(sub_adapters=None):
    global _NC_LAST
    if sub_adapters is None:
        return _NC_LAST
    key = tuple(sub_adapters)
    if key not in _NC_CACHE:
        _NC_CACHE[key] = build_bass(key)
    _NC_LAST = _NC_CACHE[key]
    return _NC_LAST


def _q8(a):
    return np.asarray(a, dtype=np.float32).astype(E4)


def kernel(x, W, b, lora_A, lora_B, masks):
    x = np.ascontiguousarray(x, dtype=np.float32)
    W = np.ascontiguousarray(W, dtype=np.float32)
    b = np.ascontiguousarray(b, dtype=np.float32)
    lora_A = np.ascontiguousarray(lora_A, dtype=np.float32)
    lora_B = np.ascontiguousarray(lora_B, dtype=np.float32)
    masks = np.ascontiguousarray(masks, dtype=np.float32)

    x_flat = x.reshape(B * T, D_IN)
    route = np.argmax(masks[..., 0].reshape(N_ADAPT, B * T), axis=0)

    # --- global balancing: tokens are dealt to cores so every core gets
    # the same per-adapter capacity (the host gathers/scatters globally
    # anyway), minimizing subtile padding across the SPMD program. ---
    G = np.bincount(route, minlength=N_ADAPT)
    n_sub_g = np.maximum((G + N_CORES * SUB - 1) // (N_CORES * SUB), 1)
    n_sub = int(n_sub_g.sum())
    if n_sub % 2:  # keep supertile halves intact
        n_sub_g[int(np.argmin(n_sub_g))] += 1
        n_sub += 1
    base_sub = np.zeros(N_ADAPT, dtype=np.int64)
    base_sub[1:] = np.cumsum(n_sub_g)[:-1]
    cap = n_sub * SUB  # tokens per core (incl. padding)
    n_sup = (n_sub + N_SUB - 1) // N_SUB

    sub_adapters = []
    for g in range(N_ADAPT):
        sub_adapters += [g] * int(n_sub_g[g])
    nc = _get_nc(tuple(sub_adapters))

    # token assignment: adapter-g tokens split into 8 near-equal chunks
    src_tokens = [[] for _ in range(N_CORES)]  # global token idx per core
    dst_slots = [[] for _ in range(N_CORES)]   # matching slot per core
    for g in range(N_ADAPT):
        idx_g = np.nonzero(route == g)[0]
        base = len(idx_g) // N_CORES
        extra = len(idx_g) % N_CORES
        off = 0
        for c in range(N_CORES):
            cnt = base + (1 if c < extra else 0)
            src_tokens[c].append(idx_g[off : off + cnt])
            dst_slots[c].append(base_sub[g] * SUB + np.arange(cnt))
            off += cnt

    # --- fused per-adapter weights, split into fp8 hi + scaled residual ---
    A_flat = lora_A.reshape(NR, D_IN)
    B_flat = lora_B.transpose(1, 0, 2).reshape(D_OUT, NR)
    w8s, w32s = [], []
    for g in range(N_ADAPT):
        W_eff = W + np.float32(SCALING) * (
            B_flat[:, g * R : (g + 1) * R] @ A_flat[g * R : (g + 1) * R, :]
        )
        wt = np.ascontiguousarray(W_eff.T)
        W8 = _q8(wt)
        W32 = _q8(WSCALE * (wt - W8.astype(np.float32)))
        w8s.append(W8.reshape(KC, P, D_OUT))
        w32s.append(W32.reshape(KC, P, D_OUT))
    wcat = np.stack(
        [np.concatenate([w8s[g], w32s[g]], axis=0) for g in range(N_ADAPT)],
        axis=0,
    ).reshape(N_ADAPT * 2 * KC * P, D_OUT)

    b16 = b.astype(np.float16)

    # --- per-core token gather + fp8 reps ---
    sup_cap = n_sup * SUP  # x dram capacity (last supertile zero-padded)
    in_maps = []
    srcs, dsts = [], []
    for c in range(N_CORES):
        src = np.concatenate(src_tokens[c])
        dst = np.concatenate(dst_slots[c]).astype(np.int64)
        srcs.append(src)
        dsts.append(dst)

        x_perm = np.zeros((sup_cap, D_IN), dtype=np.float32)
        x_perm[dst] = x_flat[src]

        x8 = _q8(x_perm)
        x8f = x8.astype(np.float32)
        xlo = _q8(x_perm - x8f)
        x8s = _q8(x8f / WSCALE)
        reps = []
        for arr in (x8, xlo, x8s):
            shard = arr.reshape(n_sup, SUP, KC, P)
            reps.append(shard.transpose(0, 3, 2, 1))  # [s, p, k, t]
        xcat = np.stack(reps, axis=2)  # [s, p, rep, k, t]
        xcat = np.ascontiguousarray(xcat).reshape(n_sup * P, NREP * KC * SUP)
        in_maps.append({"xcat": xcat, "wcat": wcat, "bias": b16})

    res = run_bass_kernel_spmd(nc, in_maps, core_ids=list(range(N_CORES)))
    out = np.empty((B * T, D_OUT), dtype=np.float32)
    for c in range(N_CORES):
        o = np.asarray(res.results[c]["out"], dtype=np.float32)
        out[srcs[c]] = o[dsts[c]]
    return out.reshape(B, T, D_OUT)


# revision 34
# speedup vs baseline: 1.7135x; 1.0031x over previous
"""Routed-LoRA linear layer (moe_routing) on 8 trn2 NeuronCores.

Math (per token t):
  out[t, :] = W @ x[t] + b + 2.0 * sum_n mask[n, t] * (B_n @ (A_n @ x[t]))

Strategy (v3, fused per-adapter weights + fp8 DoubleRow):
  - The routing is one-hot per token, so the whole layer collapses to
        out[t] = W_eff[route(t)] @ x[t] + b,   W_eff_n = W + 2 * B_n @ A_n
    with the four W_eff_n precomputed on the host. Tokens are sorted by
    adapter on the host (gather), so every 128-token matmul subtile uses a
    single W_eff_n; the host scatters the output back to token order.
  - Data-parallel over B*T = 65536 tokens: 8192 tokens per core. Each
    adapter group is padded to a multiple of 128 tokens; group capacities
    are the max over cores so all cores share one SPMD program. The
    subtile->adapter map depends on the routing counts, so the bass
    program is built (and cached) per counts signature.
  - The dense GEMM runs as THREE fp8e4m3 DoubleRow passes accumulated in
    one PSUM group:
        x8 @ W8  +  xlo @ W8  +  (x8/32)_q @ (32*(W_eff - W8))_q
    where x8 = q8(x), xlo = q8(x - x8) (kills the x-quantization error),
    and the third term is a scale-ridden W-residual correction (the W
    residual ~1e-3 sits below e4m3's subnormal floor, so it is scaled up
    32x and the 1/32 rides in a pre-scaled copy of x8). All three terms
    land at the correct absolute scale, so they share one PSUM group.
    Measured max-rel error of this scheme is ~1.5e-3.
  - DoubleRow contracts two 128-row k-tiles per instruction at half the
    per-row cost, so each pass is 4x cheaper than an fp32r pass.
  - Eviction: Activation-engine PSUM->SBUF fp16 copy, then a DVE fp16
    in-place bias add. Output DMA'd as fp16; the host casts to fp32.
"""

import numpy as np
import ml_dtypes

import concourse.bass as bass
from concourse import bacc
import concourse.mybir as mybir
import concourse.tile as tile
from concourse.bass_utils import run_bass_kernel_spmd

N_CORES = 8
B, T = 8, 8192
D_IN = 1024
D_OUT = 1024
N_ADAPT, R = 4, 16
NR = N_ADAPT * R
SCALING = 32.0 / 16.0

TOK = B * T // N_CORES  # 8192 tokens per core
SUP = 512               # tokens per supertile
SUB = 128               # tokens per matmul M-tile
N_SUB = SUP // SUB      # 4
P = 128
KC = D_IN // P          # 8 contraction chunks
NB = D_OUT // 512       # 2 PSUM-bank column halves
NREP = 3                # x8, xlo, x8s
WSCALE = 32.0

F32 = mybir.dt.float32
F16 = mybir.dt.float16
F8 = mybir.dt.float8e4
DR = mybir.MatmulPerfMode.DoubleRow

E4 = ml_dtypes.float8_e4m3


def build_bass(sub_adapters):
    """sub_adapters: tuple of adapter ids, one per 128-token subtile;
    length must be even (the last supertile may hold 2 subtiles)."""
    n_sub = len(sub_adapters)
    assert n_sub % 2 == 0
    n_sup = (n_sub + N_SUB - 1) // N_SUB
    rem_last = n_sub - N_SUB * (n_sup - 1)
    cap = n_sub * SUB

    nc = bacc.Bacc(
        "TRN2", target_bir_lowering=False, debug=False, num_devices=N_CORES
    )

    xcat_d = nc.dram_tensor(
        "xcat", [n_sup * P, NREP * KC * SUP], F8, kind="ExternalInput"
    )
    # wcat rows: ((g*2 + grp)*KC + kc, p); cols: n.  grp 0 = W8, 1 = W32
    wcat_d = nc.dram_tensor(
        "wcat", [N_ADAPT * 2 * KC * P, D_OUT], F8, kind="ExternalInput"
    )
    bias_d = nc.dram_tensor("bias", [D_OUT], F16, kind="ExternalInput")
    out_d = nc.dram_tensor("out", [cap, D_OUT], F16, kind="ExternalOutput")

    xcat_r = xcat_d.ap().rearrange("(s p) (c t) -> s p c t", p=P, t=SUP)
    wcat_r = wcat_d.ap().rearrange("(c p) n -> p c n", p=P)
    out_half = out_d.ap().rearrange("(h q p) n -> h p q n", q=2, p=P)
    out_sub = out_d.ap().rearrange("(qq p) n -> qq p n", p=P)
    bias_bcast = bass.AP(tensor=bias_d, offset=0, ap=[[0, P], [1, D_OUT]])

    # distinct adapters in first-use order, for W preload scheduling
    first_use = []
    for a in sub_adapters:
        if a not in first_use:
            first_use.append(a)
    for a in range(N_ADAPT):
        if a not in first_use:
            first_use.append(a)
    a0 = first_use[0]

    with tile.TileContext(nc) as tc:
        with (
            tc.tile_pool(name="const", bufs=1) as const,
            tc.tile_pool(name="xp", bufs=4) as xp,
            tc.tile_pool(name="op", bufs=2) as op,
            tc.tile_pool(name="pso", bufs=8, space="PSUM") as pso,
        ):
            w_sb = const.tile([P, N_ADAPT * 2 * KC, D_OUT], F8)
            b_sb = const.tile([P, D_OUT], F16)

            def wslice(g, grp, c):
                return (g * 2 + grp) * KC + c

            # Startup-critical loads on the sync queue in exact first-use
            # order (single queue => FIFO through the DGE => DMA device
            # drains in this order): x8(s0) gates pass 0 with W8[a0],
            # xlo(s0) pass 1, W32[a0] + x8s(s0) pass 2.
            # PE p-state pre-warm: dummy matmuls on a zeroed scratch row
            # keep the tensor engine continuously busy from t~0 until past
            # the startup-data arrival, so the 3us ramp to 2.4 GHz is paid
            # during the DMA wait and the real matmuls start at full rate.
            dm = const.tile([1, 640], F8)
            nc.vector.memset(dm[:], 0.0)
            d_ps = pso.tile([P, 512], F32, tag="ops", name="warm")
            for _ in range(7):
                nc.tensor.matmul(
                    d_ps[:], dm[0:1, 0:P], dm[0:1, P : P + 512],
                    start=True, stop=True, skip_group_check=True,
                )
            for _ in range(60):
                nc.tensor.matmul(
                    d_ps[:, 0:64], dm[0:1, 0:P], dm[0:1, P : P + 64],
                    start=True, stop=True, skip_group_check=True,
                )

            x_sb0 = xp.tile([P, NREP * KC, SUP], F8, tag="x")
            nc.sync.dma_start(out=x_sb0[:, :KC, :], in_=xcat_r[0][:, :KC, :])
            for c in range(KC):
                nc.sync.dma_start(
                    out=w_sb[:, wslice(a0, 0, c), :],
                    in_=wcat_r[:, wslice(a0, 0, c), :],
                )
            nc.sync.dma_start(
                out=x_sb0[:, KC : 2 * KC, :], in_=xcat_r[0][:, KC : 2 * KC, :]
            )
            for c in range(KC):
                nc.sync.dma_start(
                    out=w_sb[:, wslice(a0, 1, c), :],
                    in_=wcat_r[:, wslice(a0, 1, c), :],
                )
            nc.sync.dma_start(
                out=x_sb0[:, 2 * KC :, :], in_=xcat_r[0][:, 2 * KC :, :]
            )
            nc.gpsimd.dma_start(out=b_sb[:], in_=bias_bcast)
            # Remaining adapters' W chunks trickle in 4-per-supertile (on
            # the gpsimd queue) so they never displace the x streams on
            # the DMA device during the first supertiles.
            pending_w = [
                (g, grp, c)
                for g in first_use[1:]
                for grp in range(2)
                for c in range(KC)
            ]

            for s in range(n_sup):
                if s == 0:
                    x_sb = x_sb0
                else:
                    x_sb = xp.tile([P, NREP * KC, SUP], F8, tag="x")
                    nc.sync.dma_start(
                        out=x_sb[:, : 2 * KC, :], in_=xcat_r[s][:, : 2 * KC, :]
                    )
                    nc.sync.dma_start(
                        out=x_sb[:, 2 * KC :, :], in_=xcat_r[s][:, 2 * KC :, :]
                    )
                # sync queue, right after this supertile's x loads: the xp
                # pool's WAR dependency paces the FIFO queue at one W batch
                # per supertile without depending on eviction timing.
                for g, grp, c in ([] if s == 0 else pending_w[8 * (s - 1) : 8 * s]):
                    nc.sync.dma_start(
                        out=w_sb[:, wslice(g, grp, c), :],
                        in_=wcat_r[:, wslice(g, grp, c), :],
                    )

                o_sb = op.tile([P, N_SUB, D_OUT], F16, tag="o")
                n_q = rem_last if s == n_sup - 1 else N_SUB

                def emit_group(o_ps, q, nsl, rep_grp):
                    ts = q * SUB
                    g = sub_adapters[s * N_SUB + q]
                    for rep, grp in rep_grp:
                        xo = rep * KC
                        wo = wslice(g, grp, 0)
                        for j in range(KC // 2):
                            nc.tensor.matmul(
                                o_ps[:],
                                x_sb[:, xo + 2 * j : xo + 2 * j + 2, ts : ts + SUB],
                                w_sb[:, wo + 2 * j : wo + 2 * j + 2, nsl],
                                start=(rep == 0 and j == 0),
                                stop=(rep == 2 and j == KC // 2 - 1),
                                perf_mode=DR,
                                skip_group_check=(s == 0),
                            )

                def evict(o_ps, q, nsl, last_sup):
                    # PSUM->SBUF fp16 on Act, then fp16 bias add on DVE;
                    # on the last supertile do it in one DVE op to cut the
                    # serial tail.
                    if last_sup:
                        nc.vector.tensor_add(o_sb[:, q, nsl], o_ps[:], b_sb[:, nsl])
                    else:
                        nc.scalar.copy(o_sb[:, q, nsl], o_ps[:])
                        nc.vector.tensor_add(
                            o_sb[:, q, nsl], o_sb[:, q, nsl], b_sb[:, nsl]
                        )

                if s == 0:
                    # pass-major across all 8 PSUM banks: the first pass
                    # only needs x8+W8, which land first, so the PE ramps
                    # while xlo/W32/x8s are still in flight.
                    o_pss = [
                        pso.tile([P, 512], F32, tag="ops", name=f"ops0_{i}")
                        for i in range(8)
                    ]
                    for rep_grp in ((0, 0), (1, 0), (2, 1)):
                        for q in range(N_SUB):
                            for n in range(NB):
                                nsl = slice(n * 512, (n + 1) * 512)
                                emit_group(o_pss[q * NB + n], q, nsl, [rep_grp])
                    for q in range(N_SUB):
                        for n in range(NB):
                            nsl = slice(n * 512, (n + 1) * 512)
                            evict(o_pss[q * NB + n], q, nsl, False)
                    nc.gpsimd.dma_start(out=out_half[0], in_=o_sb[:, 0:2, :])
                    nc.gpsimd.dma_start(out=out_half[1], in_=o_sb[:, 2:4, :])
                else:
                    for q in range(n_q):
                        for n in range(NB):
                            nsl = slice(n * 512, (n + 1) * 512)
                            o_ps = pso.tile([P, 512], F32, tag="ops")
                            emit_group(
                                o_ps, q, nsl, [(0, 0), (1, 0), (2, 1)]
                            )
                            evict(o_ps, q, nsl, s == n_sup - 1)
                        if s == n_sup - 1:
                            # last supertile: per-subtile out DMA (tail)
                            nc.gpsimd.dma_start(
                                out=out_sub[s * N_SUB + q], in_=o_sb[:, q, :]
                            )
                        elif q == 1:
                            nc.gpsimd.dma_start(
                                out=out_half[2 * s], in_=o_sb[:, 0:2, :]
                            )
                    if s != n_sup - 1:
                        nc.gpsimd.dma_start(
                            out=out_half[2 * s + 1], in_=o_sb[:, 2:4, :]
                        )

    nc.compile()
    return nc


_NC_CACHE = {}
_NC_LAST = None


def _get_nc(sub_adapters=None):
    global _NC_LAST
    if sub_adapters is None:
        return _NC_LAST
    key = tuple(sub_adapters)
    if key not in _NC_CACHE:
        _NC_CACHE[key] = build_bass(key)
    _NC_LAST = _NC_CACHE[key]
    return _NC_LAST


def _q8(a):
    return np.asarray(a, dtype=np.float32).astype(E4)


def kernel(x, W, b, lora_A, lora_B, masks):
    x = np.ascontiguousarray(x, dtype=np.float32)
    W = np.ascontiguousarray(W, dtype=np.float32)
    b = np.ascontiguousarray(b, dtype=np.float32)
    lora_A = np.ascontiguousarray(lora_A, dtype=np.float32)
    lora_B = np.ascontiguousarray(lora_B, dtype=np.float32)
    masks = np.ascontiguousarray(masks, dtype=np.float32)

    x_flat = x.reshape(B * T, D_IN)
    route = np.argmax(masks[..., 0].reshape(N_ADAPT, B * T), axis=0)

    # --- global balancing: tokens are dealt to cores so every core gets
    # the same per-adapter capacity (the host gathers/scatters globally
    # anyway), minimizing subtile padding across the SPMD program. ---
    G = np.bincount(route, minlength=N_ADAPT)
    n_sub_g = np.maximum((G + N_CORES * SUB - 1) // (N_CORES * SUB), 1)
    n_sub = int(n_sub_g.sum())
    if n_sub % 2:  # keep supertile halves intact
        n_sub_g[int(np.argmin(n_sub_g))] += 1
        n_sub += 1
    base_sub = np.zeros(N_ADAPT, dtype=np.int64)
    base_sub[1:] = np.cumsum(n_sub_g)[:-1]
    cap = n_sub * SUB  # tokens per core (incl. padding)
    n_sup = (n_sub + N_SUB - 1) // N_SUB

    sub_adapters = []
    for g in range(N_ADAPT):
        sub_adapters += [g] * int(n_sub_g[g])
    nc = _get_nc(tuple(sub_adapters))

    # token assignment: adapter-g tokens split into 8 near-equal chunks
    src_tokens = [[] for _ in range(N_CORES)]  # global token idx per core
    dst_slots = [[] for _ in range(N_CORES)]   # matching slot per core
    for g in range(N_ADAPT):
        idx_g = np.nonzero(route == g)[0]
        base = len(idx_g) // N_CORES
        extra = len(idx_g) % N_CORES
        off = 0
        for c in range(N_CORES):
            cnt = base + (1 if c < extra else 0)
            src_tokens[c].append(idx_g[off : off + cnt])
            dst_slots[c].append(base_sub[g] * SUB + np.arange(cnt))
            off += cnt

    # --- fused per-adapter weights, split into fp8 hi + scaled residual ---
    A_flat = lora_A.reshape(NR, D_IN)
    B_flat = lora_B.transpose(1, 0, 2).reshape(D_OUT, NR)
    w8s, w32s = [], []
    for g in range(N_ADAPT):
        W_eff = W + np.float32(SCALING) * (
            B_flat[:, g * R : (g + 1) * R] @ A_flat[g * R : (g + 1) * R, :]
        )
        wt = np.ascontiguousarray(W_eff.T)
        W8 = _q8(wt)
        W32 = _q8(WSCALE * (wt - W8.astype(np.float32)))
        w8s.append(W8.reshape(KC, P, D_OUT))
        w32s.append(W32.reshape(KC, P, D_OUT))
    wcat = np.stack(
        [np.concatenate([w8s[g], w32s[g]], axis=0) for g in range(N_ADAPT)],
        axis=0,
    ).reshape(N_ADAPT * 2 * KC * P, D_OUT)

    b16 = b.astype(np.float16)

    # --- per-core token gather + fp8 reps ---
    sup_cap = n_sup * SUP  # x dram capacity (last supertile zero-padded)
    in_maps = []
    srcs, dsts = [], []
    for c in range(N_CORES):
        src = np.concatenate(src_tokens[c])
        dst = np.concatenate(dst_slots[c]).astype(np.int64)
        srcs.append(src)
        dsts.append(dst)

        x_perm = np.zeros((sup_cap, D_IN), dtype=np.float32)
        x_perm[dst] = x_flat[src]

        x8 = _q8(x_perm)
        x8f = x8.astype(np.float32)
        xlo = _q8(x_perm - x8f)
        x8s = _q8(x8f / WSCALE)
        reps = []
        for arr in (x8, xlo, x8s):
            shard = arr.reshape(n_sup, SUP, KC, P)
            reps.append(shard.transpose(0, 3, 2, 1))  # [s, p, k, t]
        xcat = np.stack(reps, axis=2)  # [s, p, rep, k, t]
        xcat = np.ascontiguousarray(xcat).reshape(n_sup * P, NREP * KC * SUP)
        in_maps.append({"xcat": xcat, "wcat": wcat, "bias": b16})

    res = run_bass_kernel_spmd(nc, in_maps, core_ids=list(range(N_CORES)))
    out = np.empty((B * T, D_OUT), dtype=np.float32)
    for c in range(N_CORES):
        o = np.asarray(res.results[c]["out"], dtype=np.float32)
        out[srcs[c]] = o[dsts[c]]
    return out.reshape(B, T, D_OUT)
